# revision 15
# baseline (speedup 1.0000x reference)
"""BirthDeathAttention kernel for 8 Trainium2 NeuronCores.

Math note: in the reference, both `persistence_bias` ([1,H,1,1]) and
`importance_weights[:, None, :, None] * 0.1` ([B,1,N,1]) are constant along
the softmax (key) axis, so they cancel exactly inside the softmax.  The
module is therefore plain multi-head attention + output projection.

Sharding (per the tensor-parallel hint): core = (batch b, head-group g),
b in {0,1}, g in {0..3}, each core handling 4 of the 16 heads for one batch
element.  Each core computes a partial output projection (its heads' slice
of W_proj rows); the host sums the 4 partials per batch and adds b_proj.

Per-core device pipeline (all matmuls bf16, accumulation fp32):
  A) qk^T = W_qk^T x^T    -> q^T,k^T channel-major [512, 2048]
  B) v    = x W_v          -> position-major [2048, 256]
  C) per (head-pair, query-block):
       S^T tiles = k^T q   (row-packed 2 heads, contraction d=64)
       E = exp(SCALE * S^T) on ScalarE (no max subtraction needed: scores
           are O(20), exp fits fp32/bf16 comfortably)
       U = v^T E^T          (col-packed 2 heads, contraction over keys)
       sums = 1^T E^T       (col-packed matmul-with-ones -> softmax denom)
       O^T = U * (1/sums)   (reciprocal + partition-broadcast + multiply)
  E) partial_out = O W_p   -> [2048, 1024] fp32, DMA to DRAM
"""

import sys

if "/opt/trn_rl_repo" not in sys.path:
    sys.path.insert(0, "/opt/trn_rl_repo")

import numpy as np
import ml_dtypes

import concourse.bass as bass
import concourse.mybir as mybir
import concourse.tile as tile
from concourse.bass_utils import run_bass_kernel_spmd

DIM = 1024
N = 2048
B = 2
HEADS = 16
HEAD_DIM = 64
SCALE = HEAD_DIM ** -0.5
HPG = 4          # heads per group (per core)
GC = HPG * HEAD_DIM  # channels per core = 256
BF16 = mybir.dt.bfloat16
F32 = mybir.dt.float32


def _split_multi_waits(nc, max_waits=1):
    """The walrus build in this container accepts at most one sync-wait per
    instruction.  Hoist extra waits onto single-wait NOPs inserted just
    before the instruction in its engine's program order (instructions on
    one engine execute in order, so an AND of waits on one instruction is
    equivalent to a chain of single-wait NOPs followed by the rest)."""
    uid = [0]
    for f in nc.m.functions:
        for bb in f.blocks:
            insts = bb.instructions
            new = []
            changed = False
            for inst in insts:
                si = inst.sync_info
                if si is not None and len(si.on_wait) > max_waits:
                    waits = list(si.on_wait)
                    for w in waits[:-max_waits]:
                        nop = mybir.InstNoOp(
                            name=f"I-splitw-{uid[0]}", ins=[], outs=[])
                        uid[0] += 1
                        nop.engine = inst.engine
                        nop.sync_info = mybir.SyncInfo(
                            on_wait=[w], on_update=[])
                        new.append(nop)
                    si.on_wait = waits[-max_waits:]
                    inst.sync_info = si
                    changed = True
                new.append(inst)
            if changed:
                bb.instructions = new


def build_core_kernel() -> bass.Bass:
    nc = bass.Bass()
    xT = nc.declare_dram_parameter("xT", [DIM, N], BF16, isOutput=False)
    wqk = nc.declare_dram_parameter("wqk", [DIM, 2 * GC], BF16, isOutput=False)
    wv = nc.declare_dram_parameter("wv", [DIM, GC], BF16, isOutput=False)
    wp = nc.declare_dram_parameter("wp", [GC, DIM], BF16, isOutput=False)
    out = nc.declare_dram_parameter("out", [N, DIM], F32, isOutput=True)

    KT = DIM // 128      # 8 contraction tiles over model dim
    NB = N // 512        # 4 query blocks
    NKT = N // 128       # 16 key tiles
    MT = N // 128        # 16 output row tiles

    xT_r = xT.rearrange("(kt p) n -> p kt n", p=128)
    wqk_r = wqk.rearrange("(kt p) c -> p kt c", p=128)
    wv_r = wv.rearrange("(kt p) c -> p kt c", p=128)
    wp_r = wp.rearrange("(pair p) c -> p pair c", p=128)

    with tile.TileContext(nc) as tc:
        from contextlib import ExitStack

        with ExitStack() as ctx:
            consts = ctx.enter_context(tc.tile_pool(name="consts", bufs=1))
            sbuf = ctx.enter_context(tc.tile_pool(name="sbuf", bufs=1))

            # --- resident SBUF tensors -------------------------------------
            xT_sb = sbuf.tile([128, KT, N], BF16, tag="xT")
            wqk_sb = consts.tile([128, KT, 2 * GC], BF16, tag="wqk")
            wv_sb = consts.tile([128, KT, GC], BF16, tag="wv")
            wp_sb = consts.tile([128, 2, DIM], BF16, tag="wp")
            ones_sb = consts.tile([128, 1], BF16, tag="ones")
            qk_sb = sbuf.tile([128, 4, N], BF16, tag="qk")
            v_sb = sbuf.tile([128, NKT, GC], BF16, tag="v")
            o_sb = sbuf.tile([128, 2, N], BF16, tag="o")

            nc.vector.memset(ones_sb[:], 1.0)
            # interleave weight and xT chunk DMAs so stage A's first
            # accumulation chains can start before everything lands
            for kt in range(KT):
                nc.sync.dma_start(out=wqk_sb[:, kt, :], in_=wqk_r[:, kt, :])
                nc.sync.dma_start(
                    out=xT_sb[:, kt, 0:512], in_=xT_r[:, kt, 0:512]
                )
            for nb in range(1, NB):
                for kt in range(KT):
                    nc.sync.dma_start(
                        out=xT_sb[:, kt, nb * 512:(nb + 1) * 512],
                        in_=xT_r[:, kt, nb * 512:(nb + 1) * 512],
                    )
            for kt in range(KT):
                nc.sync.dma_start(out=wv_sb[:, kt, :], in_=wv_r[:, kt, :])
            for pair in range(2):
                nc.sync.dma_start(out=wp_sb[:, pair, :], in_=wp_r[:, pair, :])

            # --- stage A: q^T,k^T (channel-major) --------------------------
            # k c-tiles (2,3) for all blocks first, then q (0,1), so stage
            # C's first exp can start as early as possible.
            with tc.tile_pool(name="psA", bufs=2, space="PSUM") as psA:
                for phase in ((2, 3), (0, 1)):
                    for nb in range(NB):
                        for ct in phase:
                            acc = psA.tile([128, 512], F32, tag="psA")
                            for kt in range(KT):
                                nc.tensor.matmul(
                                    acc[:],
                                    lhsT=wqk_sb[:, kt, ct * 128:(ct + 1) * 128],
                                    rhs=xT_sb[:, kt, nb * 512:(nb + 1) * 512],
                                    start=(kt == 0),
                                    stop=(kt == KT - 1),
                                )
                            nc.vector.tensor_copy(
                                qk_sb[:, ct, nb * 512:(nb + 1) * 512], acc[:]
                            )

            # --- stage B: v (position-major) -------------------------------
            with tc.tile_pool(name="psB", bufs=2, space="PSUM") as psB:
                for nt in range(NKT):
                    acc = psB.tile([128, GC], F32, tag="psB")
                    for kt in range(KT):
                        nc.tensor.matmul(
                            acc[:],
                            lhsT=xT_sb[:, kt, nt * 128:(nt + 1) * 128],
                            rhs=wv_sb[:, kt, :],
                            start=(kt == 0),
                            stop=(kt == KT - 1),
                        )
                    nc.vector.tensor_copy(v_sb[:, nt, :], acc[:])

            # --- stage C: attention (+ stage E interleaved per query block)
            with (
                tc.tile_pool(name="psS", bufs=2, space="PSUM") as psS,
                tc.tile_pool(name="psU", bufs=1, space="PSUM") as psU,
                tc.tile_pool(name="psR", bufs=1, space="PSUM") as psR,
                tc.tile_pool(name="psE", bufs=2, space="PSUM") as psE,
                tc.tile_pool(name="epool", bufs=2) as epool,
                tc.tile_pool(name="rpool", bufs=2) as rpool,
                tc.tile_pool(name="rdram", bufs=2, space="DRAM") as rdram,
                tc.tile_pool(name="opool", bufs=3) as opool,
            ):
                for nqb in range(NB):
                    for pair in range(2):
                        qt = qk_sb[:, pair, :]
                        kt_sb = qk_sb[:, 2 + pair, :]
                        e_t = epool.tile([128, NKT, 1024], BF16, tag="e")
                        # S^T tiles + exp
                        for nkt in range(NKT):
                            st = psS.tile([128, 1024], F32, tag="st")
                            for hh in range(2):
                                nc.tensor.matmul(
                                    st[:, hh * 512:(hh + 1) * 512],
                                    lhsT=kt_sb[
                                        hh * 64:(hh + 1) * 64,
                                        nkt * 128:(nkt + 1) * 128,
                                    ],
                                    rhs=qt[
                                        hh * 64:(hh + 1) * 64,
                                        nqb * 512:(nqb + 1) * 512,
                                    ],
                                    start=True,
                                    stop=True,
                                )
                            nc.scalar.activation(
                                e_t[:, nkt, :],
                                st[:],
                                mybir.ActivationFunctionType.Exp,
                                scale=SCALE,
                            )
                        # U = v^T E^T: the two heads' matmuls are emitted
                        # back-to-back with disjoint column groups so the PE
                        # runs them concurrently
                        u_t = psU.tile([128, 512], F32, tag="u")
                        sums_t = psR.tile([128, 512], F32, tag="sums")
                        for nkt in range(NKT):
                            for hh in range(2):
                                h = pair * 2 + hh
                                nc.tensor.matmul(
                                    u_t[hh * 64:(hh + 1) * 64, :],
                                    lhsT=v_sb[:, nkt, h * 64:(h + 1) * 64],
                                    rhs=e_t[:, nkt, hh * 512:(hh + 1) * 512],
                                    start=(nkt == 0),
                                    stop=(nkt == NKT - 1),
                                    tile_position=(0, hh * 64),
                                )
                        # softmax denominators via matmul-with-ones, 2-way
                        # column-packed (col groups 0 and 1 are disjoint)
                        for nkt in range(NKT):
                            for hh in range(2):
                                nc.tensor.matmul(
                                    sums_t[hh * 32:hh * 32 + 1, :],
                                    lhsT=ones_sb[:, 0:1],
                                    rhs=e_t[:, nkt, hh * 512:(hh + 1) * 512],
                                    start=(nkt == 0),
                                    stop=(nkt == NKT - 1),
                                    tile_position=(0, hh * 32),
                                )
                        # normalize: O^T = U / sums
                        r_t = rpool.tile([128, 512], F32, tag="r")
                        rr_t = rpool.tile([128, 512], F32, tag="rr")
                        r_dr = rdram.tile([2, 512], F32, tag="rdr")
                        # one reciprocal over rows 0..32 covers both heads'
                        # sum rows (rows 1..31 are unused garbage)
                        nc.vector.reciprocal(r_t[0:33, :], sums_t[0:33, :])
                        for hh in range(2):
                            nc.sync.dma_start(
                                out=r_dr[hh:hh + 1, :],
                                in_=r_t[hh * 32:hh * 32 + 1, :],
                            )
                            nc.sync.dma_start(
                                out=rr_t[hh * 64:(hh + 1) * 64, :],
                                in_=r_dr[hh:hh + 1, :].to_broadcast([64, 512]),
                            )
                        nc.vector.tensor_mul(
                            o_sb[:, pair, nqb * 512:(nqb + 1) * 512],
                            u_t[:],
                            rr_t[:],
                        )

                    # stage E for this query block: partial projection
                    for mt in range(nqb * 4, nqb * 4 + 4):
                        ot = opool.tile([128, DIM], F32, tag="ot")
                        for nh in range(2):
                            acc = psE.tile([128, 512], F32, tag="psE")
                            for pair in range(2):
                                nc.tensor.matmul(
                                    acc[:],
                                    lhsT=o_sb[:, pair, mt * 128:(mt + 1) * 128],
                                    rhs=wp_sb[:, pair, nh * 512:(nh + 1) * 512],
                                    start=(pair == 0),
                                    stop=(pair == 1),
                                )
                            nc.vector.tensor_copy(
                                ot[:, nh * 512:(nh + 1) * 512], acc[:]
                            )
                        nc.sync.dma_start(
                            out=out[mt * 128:(mt + 1) * 128, :], in_=ot[:]
                        )

    _split_multi_waits(nc)
    return nc


_NC_CACHE = None


def _get_nc():
    global _NC_CACHE
    if _NC_CACHE is None:
        _NC_CACHE = build_core_kernel()
    return _NC_CACHE


def kernel(x, importance_weights, W_qkv, W_proj, b_proj, persistence_bias,
           _results_hook=None):
    x = np.asarray(x)
    W_qkv = np.asarray(W_qkv, dtype=np.float32)
    W_proj = np.asarray(W_proj, dtype=np.float32)
    b_proj = np.asarray(b_proj, dtype=np.float32)

    bf = ml_dtypes.bfloat16
    Q = W_qkv[:, 0:DIM]
    K = W_qkv[:, DIM:2 * DIM]
    V = W_qkv[:, 2 * DIM:3 * DIM]

    in_maps = []
    for core in range(8):
        b, g = divmod(core, 4)
        sl = slice(g * GC, (g + 1) * GC)
        in_maps.append({
            "xT": np.ascontiguousarray(x[b].T).astype(bf),
            "wqk": np.ascontiguousarray(
                np.concatenate([Q[:, sl], K[:, sl]], axis=1)).astype(bf),
            "wv": np.ascontiguousarray(V[:, sl]).astype(bf),
            "wp": np.ascontiguousarray(W_proj[sl, :]).astype(bf),
        })

    nc = _get_nc()
    res = run_bass_kernel_spmd(nc, in_maps, list(range(8)))
    if _results_hook is not None:
        _results_hook(res)

    out = np.zeros((B, N, DIM), dtype=np.float32)
    for core in range(8):
        b = core // 4
        out[b] += res.results[core]["out"]
    out += b_proj[None, None, :]
    return out


# revision 19
# speedup vs baseline: 1.2738x; 1.2738x over previous
"""BirthDeathAttention kernel for 8 Trainium2 NeuronCores.

Math note: in the reference, both `persistence_bias` ([1,H,1,1]) and
`importance_weights[:, None, :, None] * 0.1` ([B,1,N,1]) are constant along
the softmax (key) axis, so they cancel exactly inside the softmax.  The
module is therefore plain multi-head attention + output projection.

Sharding (per the tensor-parallel hint): core = (batch b, head-group g),
b in {0,1}, g in {0..3}, each core handling 4 of the 16 heads for one batch
element.  Each core computes a partial output projection (its heads' slice
of W_proj rows); the host sums the 4 partials per batch and adds b_proj.

Per-core device pipeline (all matmuls bf16, accumulation fp32):
  A) qk^T = W_qk^T x^T    -> q^T,k^T channel-major [512, 2048]
  B) v    = x W_v          -> position-major [2048, 256]
  C) per (head-pair, query-block):
       S^T tiles = k^T q   (row-packed 2 heads, contraction d=64)
       E = exp(SCALE * S^T) on ScalarE (no max subtraction needed: scores
           are O(20), exp fits fp32/bf16 comfortably)
       U = v^T E^T          (col-packed 2 heads, contraction over keys)
       sums = 1^T E^T       (col-packed matmul-with-ones -> softmax denom)
       O^T = U * (1/sums)   (reciprocal + partition-broadcast + multiply)
  E) partial_out = O W_p   -> [2048, 1024] fp32, DMA to DRAM
"""

import sys

if "/opt/trn_rl_repo" not in sys.path:
    sys.path.insert(0, "/opt/trn_rl_repo")

import numpy as np
import ml_dtypes

import concourse.bass as bass
import concourse.mybir as mybir
import concourse.tile as tile
from concourse.bass_utils import run_bass_kernel_spmd

DIM = 1024
N = 2048
B = 2
HEADS = 16
HEAD_DIM = 64
SCALE = HEAD_DIM ** -0.5
HPG = 4          # heads per group (per core)
GC = HPG * HEAD_DIM  # channels per core = 256
BF16 = mybir.dt.bfloat16
F32 = mybir.dt.float32


def _split_multi_waits(nc, max_waits=1):
    """The walrus build in this container accepts at most one sync-wait per
    instruction.  Hoist extra waits onto single-wait NOPs inserted just
    before the instruction in its engine's program order (instructions on
    one engine execute in order, so an AND of waits on one instruction is
    equivalent to a chain of single-wait NOPs followed by the rest)."""
    uid = [0]
    for f in nc.m.functions:
        for bb in f.blocks:
            insts = bb.instructions
            new = []
            changed = False
            for inst in insts:
                si = inst.sync_info
                if si is not None and len(si.on_wait) > max_waits:
                    waits = list(si.on_wait)
                    for w in waits[:-max_waits]:
                        nop = mybir.InstNoOp(
                            name=f"I-splitw-{uid[0]}", ins=[], outs=[])
                        uid[0] += 1
                        nop.engine = inst.engine
                        nop.sync_info = mybir.SyncInfo(
                            on_wait=[w], on_update=[])
                        new.append(nop)
                    si.on_wait = waits[-max_waits:]
                    inst.sync_info = si
                    changed = True
                new.append(inst)
            if changed:
                bb.instructions = new


def build_core_kernel() -> bass.Bass:
    nc = bass.Bass()
    xT = nc.declare_dram_parameter("xT", [DIM, N], BF16, isOutput=False)
    wqk = nc.declare_dram_parameter("wqk", [DIM, 2 * GC], BF16, isOutput=False)
    wv = nc.declare_dram_parameter("wv", [DIM, GC], BF16, isOutput=False)
    wp = nc.declare_dram_parameter("wp", [GC, DIM], BF16, isOutput=False)
    out = nc.declare_dram_parameter("out", [N, DIM], BF16, isOutput=True)

    KT = DIM // 128      # 8 contraction tiles over model dim
    NB = N // 512        # 4 query blocks
    NKT = N // 128       # 16 key tiles
    MT = N // 128        # 16 output row tiles

    xT_r = xT.rearrange("(kt p) n -> p kt n", p=128)
    wqk_r = wqk.rearrange("(kt p) c -> p kt c", p=128)
    wv_r = wv.rearrange("(kt p) c -> p kt c", p=128)
    wp_r = wp.rearrange("(pair p) c -> p pair c", p=128)

    with tile.TileContext(nc) as tc:
        from contextlib import ExitStack

        with ExitStack() as ctx:
            consts = ctx.enter_context(tc.tile_pool(name="consts", bufs=1))
            sbuf = ctx.enter_context(tc.tile_pool(name="sbuf", bufs=1))

            # --- resident SBUF tensors -------------------------------------
            xT_sb = sbuf.tile([128, KT, N], BF16, tag="xT")
            wqk_sb = consts.tile([128, KT, 2 * GC], BF16, tag="wqk")
            wv_sb = consts.tile([128, KT, GC], BF16, tag="wv")
            wp_sb = consts.tile([128, 2, DIM], BF16, tag="wp")
            ones_sb = consts.tile([128, 1], BF16, tag="ones")
            qk_sb = sbuf.tile([128, 4, N], BF16, tag="qk")
            v_sb = sbuf.tile([128, NKT, GC], BF16, tag="v")
            o_sb = sbuf.tile([128, 2, N], BF16, tag="o")

            nc.vector.memset(ones_sb[:], 1.0)
            # interleave weight and xT chunk DMAs so stage A's first
            # accumulation chains can start before everything lands
            for kt in range(KT):
                nc.sync.dma_start(out=wqk_sb[:, kt, :], in_=wqk_r[:, kt, :])
                nc.sync.dma_start(
                    out=xT_sb[:, kt, 0:512], in_=xT_r[:, kt, 0:512]
                )
            for nb in range(1, NB):
                for kt in range(KT):
                    nc.sync.dma_start(
                        out=xT_sb[:, kt, nb * 512:(nb + 1) * 512],
                        in_=xT_r[:, kt, nb * 512:(nb + 1) * 512],
                    )
            for kt in range(KT):
                nc.sync.dma_start(out=wv_sb[:, kt, :], in_=wv_r[:, kt, :])
            for pair in range(2):
                nc.sync.dma_start(out=wp_sb[:, pair, :], in_=wp_r[:, pair, :])

            # --- stage A: q^T,k^T (channel-major) --------------------------
            # k c-tiles (2,3) for all blocks first, then q (0,1), so stage
            # C's first exp can start as early as possible.
            with tc.tile_pool(name="psA", bufs=2, space="PSUM") as psA:
                for phase in ((2, 3), (0, 1)):
                    for nb in range(NB):
                        for ct in phase:
                            acc = psA.tile([128, 512], F32, tag="psA")
                            for kt in range(KT):
                                nc.tensor.matmul(
                                    acc[:],
                                    lhsT=wqk_sb[:, kt, ct * 128:(ct + 1) * 128],
                                    rhs=xT_sb[:, kt, nb * 512:(nb + 1) * 512],
                                    start=(kt == 0),
                                    stop=(kt == KT - 1),
                                )
                            nc.vector.tensor_copy(
                                qk_sb[:, ct, nb * 512:(nb + 1) * 512], acc[:]
                            )

            # --- stage B: v (position-major) -------------------------------
            with tc.tile_pool(name="psB", bufs=2, space="PSUM") as psB:
                for nt in range(NKT):
                    acc = psB.tile([128, GC], F32, tag="psB")
                    for kt in range(KT):
                        nc.tensor.matmul(
                            acc[:],
                            lhsT=xT_sb[:, kt, nt * 128:(nt + 1) * 128],
                            rhs=wv_sb[:, kt, :],
                            start=(kt == 0),
                            stop=(kt == KT - 1),
                        )
                    nc.vector.tensor_copy(v_sb[:, nt, :], acc[:])

            # --- stage C: attention ---------------------------------------
            with (
                tc.tile_pool(name="psS", bufs=2, space="PSUM") as psS,
                tc.tile_pool(name="psU", bufs=2, space="PSUM") as psU,
                tc.tile_pool(name="psR", bufs=2, space="PSUM") as psR,
                tc.tile_pool(name="epool", bufs=2) as epool,
                tc.tile_pool(name="rpool", bufs=2) as rpool,
                tc.tile_pool(name="rdram", bufs=2, space="DRAM") as rdram,
            ):
                for nqb in range(NB):
                    for pair in range(2):
                        qt = qk_sb[:, pair, :]
                        kt_sb = qk_sb[:, 2 + pair, :]
                        e_t = epool.tile([128, NKT, 1024], BF16, tag="e")
                        # S^T tiles + exp
                        for nkt in range(NKT):
                            st = psS.tile([128, 1024], F32, tag="st")
                            for hh in range(2):
                                nc.tensor.matmul(
                                    st[:, hh * 512:(hh + 1) * 512],
                                    lhsT=kt_sb[
                                        hh * 64:(hh + 1) * 64,
                                        nkt * 128:(nkt + 1) * 128,
                                    ],
                                    rhs=qt[
                                        hh * 64:(hh + 1) * 64,
                                        nqb * 512:(nqb + 1) * 512,
                                    ],
                                    start=True,
                                    stop=True,
                                )
                            nc.scalar.activation(
                                e_t[:, nkt, :],
                                st[:],
                                mybir.ActivationFunctionType.Exp,
                                scale=SCALE,
                            )
                        # U = v^T E^T: the two heads' matmuls are emitted
                        # back-to-back with disjoint column groups so the PE
                        # runs them concurrently
                        u_t = psU.tile([128, 512], F32, tag="u")
                        sums_t = psR.tile([128, 512], F32, tag="sums")
                        for nkt in range(NKT):
                            for hh in range(2):
                                h = pair * 2 + hh
                                nc.tensor.matmul(
                                    u_t[hh * 64:(hh + 1) * 64, :],
                                    lhsT=v_sb[:, nkt, h * 64:(h + 1) * 64],
                                    rhs=e_t[:, nkt, hh * 512:(hh + 1) * 512],
                                    start=(nkt == 0),
                                    stop=(nkt == NKT - 1),
                                    tile_position=(0, hh * 64),
                                )
                        # softmax denominators via matmul-with-ones, 2-way
                        # column-packed (col groups 0 and 1 are disjoint)
                        for nkt in range(NKT):
                            for hh in range(2):
                                nc.tensor.matmul(
                                    sums_t[hh * 32:hh * 32 + 1, :],
                                    lhsT=ones_sb[:, 0:1],
                                    rhs=e_t[:, nkt, hh * 512:(hh + 1) * 512],
                                    start=(nkt == 0),
                                    stop=(nkt == NKT - 1),
                                    tile_position=(0, hh * 32),
                                )
                        # normalize: O^T = U / sums
                        r_t = rpool.tile([128, 512], F32, tag="r")
                        rr_t = rpool.tile([128, 512], F32, tag="rr")
                        r_dr = rdram.tile([2, 512], F32, tag="rdr")
                        # one reciprocal over rows 0..32 covers both heads'
                        # sum rows (rows 1..31 are unused garbage)
                        nc.vector.reciprocal(r_t[0:33, :], sums_t[0:33, :])
                        for hh in range(2):
                            nc.sync.dma_start(
                                out=r_dr[hh:hh + 1, :],
                                in_=r_t[hh * 32:hh * 32 + 1, :],
                            )
                            nc.sync.dma_start(
                                out=rr_t[hh * 64:(hh + 1) * 64, :],
                                in_=r_dr[hh:hh + 1, :].to_broadcast([64, 512]),
                            )
                        nc.vector.tensor_mul(
                            o_sb[:, pair, nqb * 512:(nqb + 1) * 512],
                            u_t[:],
                            rr_t[:],
                        )

            # --- stage E: partial projection ------------------------------
            with (
                tc.tile_pool(name="psE", bufs=2, space="PSUM") as psE,
                tc.tile_pool(name="opool", bufs=3) as opool,
            ):
                for mt in range(MT):
                    ot = opool.tile([128, DIM], BF16, tag="ot")
                    for nh in range(2):
                        acc = psE.tile([128, 512], F32, tag="psE")
                        for pair in range(2):
                            nc.tensor.matmul(
                                acc[:],
                                lhsT=o_sb[:, pair, mt * 128:(mt + 1) * 128],
                                rhs=wp_sb[:, pair, nh * 512:(nh + 1) * 512],
                                start=(pair == 0),
                                stop=(pair == 1),
                            )
                        nc.vector.tensor_copy(
                            ot[:, nh * 512:(nh + 1) * 512], acc[:]
                        )
                    nc.sync.dma_start(
                        out=out[mt * 128:(mt + 1) * 128, :], in_=ot[:]
                    )

    _split_multi_waits(nc)
    return nc


_NC_CACHE = None


def _get_nc():
    global _NC_CACHE
    if _NC_CACHE is None:
        _NC_CACHE = build_core_kernel()
    return _NC_CACHE


def kernel(x, importance_weights, W_qkv, W_proj, b_proj, persistence_bias,
           _results_hook=None):
    x = np.asarray(x)
    W_qkv = np.asarray(W_qkv, dtype=np.float32)
    W_proj = np.asarray(W_proj, dtype=np.float32)
    b_proj = np.asarray(b_proj, dtype=np.float32)

    bf = ml_dtypes.bfloat16
    Q = W_qkv[:, 0:DIM]
    K = W_qkv[:, DIM:2 * DIM]
    V = W_qkv[:, 2 * DIM:3 * DIM]

    in_maps = []
    for core in range(8):
        b, g = divmod(core, 4)
        sl = slice(g * GC, (g + 1) * GC)
        in_maps.append({
            "xT": np.ascontiguousarray(x[b].T).astype(bf),
            "wqk": np.ascontiguousarray(
                np.concatenate([Q[:, sl], K[:, sl]], axis=1)).astype(bf),
            "wv": np.ascontiguousarray(V[:, sl]).astype(bf),
            "wp": np.ascontiguousarray(W_proj[sl, :]).astype(bf),
        })

    nc = _get_nc()
    res = run_bass_kernel_spmd(nc, in_maps, list(range(8)))
    if _results_hook is not None:
        _results_hook(res)

    out = np.zeros((B, N, DIM), dtype=np.float32)
    for core in range(8):
        b = core // 4
        out[b] += res.results[core]["out"].astype(np.float32)
    out += b_proj[None, None, :]
    return out


# revision 24
# speedup vs baseline: 1.3432x; 1.0544x over previous
"""BirthDeathAttention kernel for 8 Trainium2 NeuronCores.

Math note: in the reference, both `persistence_bias` ([1,H,1,1]) and
`importance_weights[:, None, :, None] * 0.1` ([B,1,N,1]) are constant along
the softmax (key) axis, so they cancel exactly inside the softmax.  The
module is therefore plain multi-head attention + output projection.

Sharding (per the tensor-parallel hint): core = (batch b, head-group g),
b in {0,1}, g in {0..3}, each core handling 4 of the 16 heads for one batch
element.  Each core computes a partial output projection (its heads' slice
of W_proj rows); the host sums the 4 partials per batch and adds b_proj.

Per-core device pipeline (all matmuls bf16, accumulation fp32):
  A) qk^T = W_qk^T x^T    -> q^T,k^T channel-major [512, 2048]
  B) v    = x W_v          -> position-major [2048, 256]
  C) per (head-pair, query-block):
       S^T tiles = k^T q   (row-packed 2 heads, contraction d=64)
       E = exp(SCALE * S^T) on ScalarE (no max subtraction needed: scores
           are O(20), exp fits fp32/bf16 comfortably)
       U = v^T E^T          (col-packed 2 heads, contraction over keys)
       sums = 1^T E^T       (col-packed matmul-with-ones -> softmax denom)
       O^T = U * (1/sums)   (reciprocal + partition-broadcast + multiply)
  E) partial_out = O W_p   -> [2048, 1024] fp32, DMA to DRAM
"""

import sys

if "/opt/trn_rl_repo" not in sys.path:
    sys.path.insert(0, "/opt/trn_rl_repo")

import numpy as np
import ml_dtypes

import concourse.bass as bass
import concourse.mybir as mybir
import concourse.tile as tile
from concourse.bass_utils import run_bass_kernel_spmd

DIM = 1024
N = 2048
B = 2
HEADS = 16
HEAD_DIM = 64
SCALE = HEAD_DIM ** -0.5
HPG = 4          # heads per group (per core)
GC = HPG * HEAD_DIM  # channels per core = 256
BF16 = mybir.dt.bfloat16
F32 = mybir.dt.float32


def _split_multi_waits(nc, max_waits=1):
    """The walrus build in this container accepts at most one sync-wait per
    instruction.  Hoist extra waits onto single-wait NOPs inserted just
    before the instruction in its engine's program order (instructions on
    one engine execute in order, so an AND of waits on one instruction is
    equivalent to a chain of single-wait NOPs followed by the rest)."""
    uid = [0]
    for f in nc.m.functions:
        for bb in f.blocks:
            insts = bb.instructions
            new = []
            changed = False
            for inst in insts:
                si = inst.sync_info
                if si is not None and len(si.on_wait) > max_waits:
                    waits = list(si.on_wait)
                    for w in waits[:-max_waits]:
                        nop = mybir.InstNoOp(
                            name=f"I-splitw-{uid[0]}", ins=[], outs=[])
                        uid[0] += 1
                        nop.engine = inst.engine
                        nop.sync_info = mybir.SyncInfo(
                            on_wait=[w], on_update=[])
                        new.append(nop)
                    si.on_wait = waits[-max_waits:]
                    inst.sync_info = si
                    changed = True
                new.append(inst)
            if changed:
                bb.instructions = new


def build_core_kernel() -> bass.Bass:
    nc = bass.Bass()
    xT = nc.declare_dram_parameter("xT", [DIM, N], BF16, isOutput=False)
    wqk = nc.declare_dram_parameter("wqk", [DIM, 2 * GC], BF16, isOutput=False)
    wv = nc.declare_dram_parameter("wv", [DIM, GC], BF16, isOutput=False)
    wp = nc.declare_dram_parameter("wp", [GC, DIM], BF16, isOutput=False)
    out = nc.declare_dram_parameter("out", [N, DIM], BF16, isOutput=True)

    KT = DIM // 128      # 8 contraction tiles over model dim
    NB = N // 512        # 4 query blocks
    NKT = N // 128       # 16 key tiles
    MT = N // 128        # 16 output row tiles

    xT_r = xT.rearrange("(kt p) n -> p kt n", p=128)
    wqk_r = wqk.rearrange("(kt p) c -> p kt c", p=128)
    wv_r = wv.rearrange("(kt p) c -> p kt c", p=128)
    wp_r = wp.rearrange("(pair p) c -> p pair c", p=128)

    with tile.TileContext(nc) as tc:
        from contextlib import ExitStack

        with ExitStack() as ctx:
            consts = ctx.enter_context(tc.tile_pool(name="consts", bufs=1))
            sbuf = ctx.enter_context(tc.tile_pool(name="sbuf", bufs=1))

            # --- resident SBUF tensors -------------------------------------
            xT_sb = sbuf.tile([128, KT, N], BF16, tag="xT")
            wqk_sb = consts.tile([128, KT, 2 * GC], BF16, tag="wqk")
            wv_sb = consts.tile([128, KT, GC], BF16, tag="wv")
            wp_sb = consts.tile([128, 2, DIM], BF16, tag="wp")
            ones_sb = consts.tile([128, 1], BF16, tag="ones")
            qk_sb = sbuf.tile([128, 4, N], BF16, tag="qk")
            # v with a ones column appended per head ([v_h | 1], stride 65):
            # the ones column turns the attention@v matmul into one that also
            # emits the softmax denominator as output row 64
            v_sb = sbuf.tile([128, NKT, HPG * 65], BF16, tag="v")
            o_sb = sbuf.tile([128, 2, N], BF16, tag="o")

            nc.vector.memset(ones_sb[:], 1.0)
            # interleave weight and xT chunk DMAs so stage A's first
            # accumulation chains can start before everything lands
            for kt in range(KT):
                nc.sync.dma_start(out=wqk_sb[:, kt, :], in_=wqk_r[:, kt, :])
                nc.sync.dma_start(
                    out=xT_sb[:, kt, 0:512], in_=xT_r[:, kt, 0:512]
                )
            for nb in range(1, NB):
                for kt in range(KT):
                    nc.sync.dma_start(
                        out=xT_sb[:, kt, nb * 512:(nb + 1) * 512],
                        in_=xT_r[:, kt, nb * 512:(nb + 1) * 512],
                    )
            for kt in range(KT):
                nc.sync.dma_start(out=wv_sb[:, kt, :], in_=wv_r[:, kt, :])
            for pair in range(2):
                nc.sync.dma_start(out=wp_sb[:, pair, :], in_=wp_r[:, pair, :])

            # --- stage A: q^T,k^T (channel-major) --------------------------
            # k c-tiles (2,3) for all blocks first, then q (0,1), so stage
            # C's first exp can start as early as possible.
            with tc.tile_pool(name="psA", bufs=2, space="PSUM") as psA:
                for phase in ((2, 3), (0, 1)):
                    for nb in range(NB):
                        for ct in phase:
                            acc = psA.tile([128, 512], F32, tag="psA")
                            for kt in range(KT):
                                nc.tensor.matmul(
                                    acc[:],
                                    lhsT=wqk_sb[:, kt, ct * 128:(ct + 1) * 128],
                                    rhs=xT_sb[:, kt, nb * 512:(nb + 1) * 512],
                                    start=(kt == 0),
                                    stop=(kt == KT - 1),
                                )
                            nc.vector.tensor_copy(
                                qk_sb[:, ct, nb * 512:(nb + 1) * 512], acc[:]
                            )

            # --- stage B: v (position-major) -------------------------------
            # ones columns (offset h*65+64) come from the initial memset
            nc.vector.memset(v_sb[:], 1.0)
            with tc.tile_pool(name="psB", bufs=2, space="PSUM") as psB:
                for nt in range(NKT):
                    acc = psB.tile([128, GC], F32, tag="psB")
                    for kt in range(KT):
                        nc.tensor.matmul(
                            acc[:],
                            lhsT=xT_sb[:, kt, nt * 128:(nt + 1) * 128],
                            rhs=wv_sb[:, kt, :],
                            start=(kt == 0),
                            stop=(kt == KT - 1),
                        )
                    for h in range(HPG):
                        nc.vector.tensor_copy(
                            v_sb[:, nt, h * 65:h * 65 + 64],
                            acc[:, h * 64:(h + 1) * 64],
                        )

            # --- stage C: attention ---------------------------------------
            with (
                tc.tile_pool(name="psS", bufs=2, space="PSUM") as psS,
                tc.tile_pool(name="psU", bufs=4, space="PSUM") as psU,
                tc.tile_pool(name="epool", bufs=2) as epool,
                tc.tile_pool(name="rpool", bufs=2) as rpool,
                tc.tile_pool(name="rdram", bufs=2, space="DRAM") as rdram,
            ):
                for nqb in range(NB):
                    for pair in range(2):
                        qt = qk_sb[:, pair, :]
                        kt_sb = qk_sb[:, 2 + pair, :]
                        e_t = epool.tile([128, NKT, 1024], BF16, tag="e")
                        # S^T tiles + exp
                        for nkt in range(NKT):
                            st = psS.tile([128, 1024], F32, tag="st")
                            for hh in range(2):
                                nc.tensor.matmul(
                                    st[:, hh * 512:(hh + 1) * 512],
                                    lhsT=kt_sb[
                                        hh * 64:(hh + 1) * 64,
                                        nkt * 128:(nkt + 1) * 128,
                                    ],
                                    rhs=qt[
                                        hh * 64:(hh + 1) * 64,
                                        nqb * 512:(nqb + 1) * 512,
                                    ],
                                    start=True,
                                    stop=True,
                                )
                            nc.scalar.activation(
                                e_t[:, nkt, :],
                                st[:],
                                mybir.ActivationFunctionType.Exp,
                                scale=SCALE,
                            )
                        # U_aug = [v|1]^T E^T per head: row 64 of each
                        # accumulator is the softmax denominator.  M=65
                        # rounds to full tile_size so LDWEIGHTS stays
                        # background-buffered (hidden behind streaming).
                        u_a = psU.tile([65, 512], F32, tag="u")
                        u_b = psU.tile([65, 512], F32, tag="u")
                        for nkt in range(NKT):
                            for hh, u_t in ((0, u_a), (1, u_b)):
                                h = pair * 2 + hh
                                nc.tensor.matmul(
                                    u_t[:],
                                    lhsT=v_sb[:, nkt, h * 65:h * 65 + 65],
                                    rhs=e_t[:, nkt, hh * 512:(hh + 1) * 512],
                                    start=(nkt == 0),
                                    stop=(nkt == NKT - 1),
                                )
                        # normalize: O^T = U / sums
                        r_in = rpool.tile([33, 512], F32, tag="rin")
                        r_t = rpool.tile([33, 512], F32, tag="r")
                        rr_t = rpool.tile([128, 512], F32, tag="rr")
                        r_dr = rdram.tile([2, 512], F32, tag="rdr")
                        nc.vector.tensor_copy(r_in[0:1, :], u_a[64:65, :])
                        nc.vector.tensor_copy(r_in[32:33, :], u_b[64:65, :])
                        # one reciprocal covers both heads' sum rows (rows
                        # 1..31 are unused garbage)
                        nc.vector.reciprocal(r_t[0:33, :], r_in[0:33, :])
                        for hh in range(2):
                            nc.sync.dma_start(
                                out=r_dr[hh:hh + 1, :],
                                in_=r_t[hh * 32:hh * 32 + 1, :],
                            )
                            nc.sync.dma_start(
                                out=rr_t[hh * 64:(hh + 1) * 64, :],
                                in_=r_dr[hh:hh + 1, :].to_broadcast([64, 512]),
                            )
                        nc.vector.tensor_mul(
                            o_sb[0:64, pair, nqb * 512:(nqb + 1) * 512],
                            u_a[0:64, :],
                            rr_t[0:64, :],
                        )
                        nc.vector.tensor_mul(
                            o_sb[64:128, pair, nqb * 512:(nqb + 1) * 512],
                            u_b[0:64, :],
                            rr_t[64:128, :],
                        )

            # --- stage E: partial projection ------------------------------
            with (
                tc.tile_pool(name="psE", bufs=2, space="PSUM") as psE,
                tc.tile_pool(name="opool", bufs=3) as opool,
            ):
                for mt in range(MT):
                    ot = opool.tile([128, DIM], BF16, tag="ot")
                    for nh in range(2):
                        acc = psE.tile([128, 512], F32, tag="psE")
                        for pair in range(2):
                            nc.tensor.matmul(
                                acc[:],
                                lhsT=o_sb[:, pair, mt * 128:(mt + 1) * 128],
                                rhs=wp_sb[:, pair, nh * 512:(nh + 1) * 512],
                                start=(pair == 0),
                                stop=(pair == 1),
                            )
                        nc.vector.tensor_copy(
                            ot[:, nh * 512:(nh + 1) * 512], acc[:]
                        )
                    nc.sync.dma_start(
                        out=out[mt * 128:(mt + 1) * 128, :], in_=ot[:]
                    )

    _split_multi_waits(nc)
    return nc


_NC_CACHE = None


def _get_nc():
    global _NC_CACHE
    if _NC_CACHE is None:
        _NC_CACHE = build_core_kernel()
    return _NC_CACHE


def kernel(x, importance_weights, W_qkv, W_proj, b_proj, persistence_bias,
           _results_hook=None):
    x = np.asarray(x)
    W_qkv = np.asarray(W_qkv, dtype=np.float32)
    W_proj = np.asarray(W_proj, dtype=np.float32)
    b_proj = np.asarray(b_proj, dtype=np.float32)

    bf = ml_dtypes.bfloat16
    Q = W_qkv[:, 0:DIM]
    K = W_qkv[:, DIM:2 * DIM]
    V = W_qkv[:, 2 * DIM:3 * DIM]

    in_maps = []
    for core in range(8):
        b, g = divmod(core, 4)
        sl = slice(g * GC, (g + 1) * GC)
        in_maps.append({
            "xT": np.ascontiguousarray(x[b].T).astype(bf),
            "wqk": np.ascontiguousarray(
                np.concatenate([Q[:, sl], K[:, sl]], axis=1)).astype(bf),
            "wv": np.ascontiguousarray(V[:, sl]).astype(bf),
            "wp": np.ascontiguousarray(W_proj[sl, :]).astype(bf),
        })

    nc = _get_nc()
    res = run_bass_kernel_spmd(nc, in_maps, list(range(8)))
    if _results_hook is not None:
        _results_hook(res)

    out = np.zeros((B, N, DIM), dtype=np.float32)
    for core in range(8):
        b = core // 4
        out[b] += res.results[core]["out"].astype(np.float32)
    out += b_proj[None, None, :]
    return out
